# revision 51
# baseline (speedup 1.0000x reference)
"""BiGCN layer kernel for 8 Trainium2 NeuronCores.

Strategy (1D column-parallel SpMM, fp16 streams, ReduceScatter epilogue):
  - Each core c owns the contraction slice n in [c*512, (c+1)*512) of all six
    adjacency matrices (3 bw + 3 fw), pre-transposed on host to [n_loc, m]
    fp16 so the contraction dim lands on SBUF partitions with no on-chip
    transposes. fp16 halves the dominant HBM traffic; its 11-bit mantissa
    matches the fp32r (TF32-like) precision class for these [0,1) values.
  - sup[r] = inps @ W[r] is computed locally per core for its n-slice only
    (no support gather: the column-parallel form needs just the local slice,
    so the streams start with zero collective dependency and PJRT launch
    skew + the kernel entry barrier hide under productive work).
  - feats^T partials (all m, summed over a direction's 3 relations directly
    in PSUM) stage to DRAM in fp16 and ReduceScatter across the 8 cores;
    RS(bw) overlaps the fw stream. Core c receives its own m-block.
  - bias+relu fuse into one scalar-engine activation (bias is per-partition
    because feats is produced transposed [h, m]); the final linear runs in
    fp32r, split so its bw half overlaps RS(fw); the residual adds an exact
    fp32 copy of inps^T. Host assembles the 8 transposed output blocks.
"""

import numpy as np

N, H, R = 4096, 512, 3
K = H // 2            # 256
NC = 8                # cores
NB = N // NC          # 512 rows (m / n_loc) per core
MC = 1024             # m-chunk width streamed per PSUM accumulation group

_BUILT = {}


def _build_nc():
    """Build (and cache) the Bass program. Identical program on all 8 cores."""
    if "nc" in _BUILT:
        return _BUILT["nc"]

    import concourse.bass as bass
    import concourse.mybir as mybir
    from concourse import bacc, tile

    f32 = mybir.dt.float32
    f32r = mybir.dt.float32r
    f16 = mybir.dt.float16
    nc = bacc.Bacc(None, num_devices=NC)

    inpsT = nc.dram_tensor("inpsT", [H, NB], f16, kind="ExternalInput")
    inpsR = nc.dram_tensor("inpsR", [H, NB], f32, kind="ExternalInput")
    adjT = nc.dram_tensor("adjT", [2 * R, NB, N], f16, kind="ExternalInput")
    wst = nc.dram_tensor("wst", [2 * R, H, K], f16, kind="ExternalInput")
    bstack = nc.dram_tensor("bstack", [4, 128, R], f32, kind="ExternalInput")
    w1 = nc.dram_tensor("w1", [H, H], f32r, kind="ExternalInput")
    b1s = nc.dram_tensor("b1s", [4, 128, 1], f32, kind="ExternalInput")
    outT = nc.dram_tensor("outT", [H, NB], f32, kind="ExternalOutput")

    HT = H // 128     # 4 h-tiles
    NT = NB // 128    # 4 n_loc tiles
    JT = H // 128     # 4 output j tiles
    NMC = N // MC     # 4 m chunks
    Relu = mybir.ActivationFunctionType.Relu
    Identity = mybir.ActivationFunctionType.Identity

    with tile.TileContext(nc) as tc:
        with (
            tc.tile_pool(name="const", bufs=1) as const,
            tc.tile_pool(name="adjp", bufs=6) as adjp,
            tc.tile_pool(name="evacp", bufs=3) as evacp,
            tc.tile_pool(name="psum", bufs=4, space=bass.MemorySpace.PSUM) as psump,
            tc.tile_pool(name="dram", bufs=1, space="DRAM") as dramp,
        ):
            # ---------------- constants into SBUF ----------------
            inpsT_sb = const.tile([128, HT, NB], f16)       # [p_h, ht, n_loc]
            nc.sync.dma_start(inpsT_sb[:], inpsT[:, :].rearrange("(t p) n -> p t n", p=128))
            wst_sb = const.tile([128, 2 * R, HT, K], f16)   # [p_h, r, ht, k]
            nc.sync.dma_start(wst_sb[:], wst[:, :, :].rearrange("r (t p) k -> p r t k", p=128))
            inpsR_sb = const.tile([128, HT, NB], f32)       # exact fp32 for residual
            nc.scalar.dma_start(inpsR_sb[:], inpsR[:, :].rearrange("(t p) n -> p t n", p=128))
            w1_sb = const.tile([128, HT, H], f32r)          # [p_h, ht, j]
            nc.scalar.dma_start(w1_sb[:], w1[:, :].rearrange("(t p) j -> p t j", p=128))
            bst_sb = const.tile([128, JT, R], f32)
            nc.scalar.dma_start(bst_sb[:], bstack[:, :, :].rearrange("t p r -> p t r"))
            b1_sb = const.tile([128, JT], f32)
            nc.scalar.dma_start(b1_sb[:], b1s[:, :, :].rearrange("t p o -> p (t o)"))

            # summed (over relations) concat bias, per (p, jt)
            bias_sb = const.tile([128, JT], f32)
            for jt in range(JT):
                nc.vector.tensor_add(
                    bias_sb[:, jt : jt + 1], bst_sb[:, jt, 0:1], bst_sb[:, jt, 1:2]
                )
                nc.vector.tensor_add(
                    bias_sb[:, jt : jt + 1], bias_sb[:, jt : jt + 1], bst_sb[:, jt, 2:3]
                )

            # ---------------- local supports: sup[r][n_loc, k] ----------------
            sup_sb = const.tile([128, 2 * R, NT, K], f16)   # [p_n, r, nt, k]
            for r in range(2 * R):
                for nt in range(NT):
                    ps = psump.tile([128, K], f32, tag="pb")
                    for ht in range(HT):
                        nc.tensor.matmul(
                            ps[:],
                            inpsT_sb[:, ht, nt * 128 : (nt + 1) * 128],
                            wst_sb[:, r, ht, :],
                            start=(ht == 0),
                            stop=(ht == HT - 1),
                        )
                    nc.vector.tensor_copy(sup_sb[:, r, nt, :], ps[:])

            # ---------------- adjacency stream + RS staging ----------------
            # One staging tensor per (direction, k-half): separate tensors keep
            # later streams' writes from serializing behind earlier collectives'
            # reads, and quarter-granular RS lets phase C start on each h-row
            # block as soon as its collective lands instead of waiting for a
            # full direction.
            stags = [
                dramp.tile([NC, 128, NB], f16, name=f"stag{q}", tag=f"stag{q}")
                for q in range(4)
            ]
            rs_out = [
                dramp.tile([1, 128, NB], f16, name=f"rs_out{q}", tag=f"rs_out{q}")
                for q in range(4)
            ]
            for dirn in range(2):                           # 0 = bw (h 0:256), 1 = fw
                for mc in range(NMC):
                    ps0 = psump.tile([128, MC], f32, tag="pb", name="ps0")  # k 0:128
                    ps1 = psump.tile([128, MC], f32, tag="pb", name="ps1")  # k 128:256
                    for ri in range(R):
                        r = dirn * R + ri
                        at = adjp.tile([128, NT, MC], f16, tag="adj")
                        nc.sync.dma_start(
                            at[:],
                            adjT[r, :, mc * MC : (mc + 1) * MC].rearrange(
                                "(t p) m -> p t m", p=128
                            ),
                        )
                        for nt in range(NT):
                            first = ri == 0 and nt == 0
                            last = ri == R - 1 and nt == NT - 1
                            for kk, ps in ((0, ps0), (1, ps1)):
                                lhsT = sup_sb[:, r, nt, kk * 128 : (kk + 1) * 128]
                                for mh in range(MC // 512):
                                    nc.tensor.matmul(
                                        ps[:, mh * 512 : (mh + 1) * 512],
                                        lhsT,
                                        at[:, nt, mh * 512 : (mh + 1) * 512],
                                        start=first,
                                        stop=last,
                                    )
                    for kk, ps in ((0, ps0), (1, ps1)):
                        ev = evacp.tile([128, MC], f16, tag="ev")
                        nc.vector.tensor_copy(ev[:], ps[:])
                        for d2 in range(MC // NB):
                            dest = (mc * MC) // NB + d2
                            nc.scalar.dma_start(
                                stags[dirn * 2 + kk][dest, :, :],
                                ev[:, d2 * NB : (d2 + 1) * NB],
                            )
                for kk in range(2):
                    q = dirn * 2 + kk
                    nc.gpsimd.collective_compute(
                        "ReduceScatter",
                        mybir.AluOpType.add,
                        replica_groups=[list(range(NC))],
                        ins=[stags[q][:].opt()],
                        outs=[rs_out[q][:].opt()],
                    )

            # ---------------- bias + relu + final linear + residual ----------------
            # The final matmul accumulates per h-row block so each block's
            # matmuls run as soon as its quarter-RS lands, overlapping the
            # remaining collectives.
            frelu_sb = const.tile([128, HT, NB], f32r)      # [p_h, ht, m_loc]
            psos = []
            for ht in range(HT):                            # ht == RS quarter q
                ft = evacp.tile([128, NB], f16, tag="ftmp")
                nc.scalar.dma_start(ft[:], rs_out[ht][0, :, :])
                nc.scalar.activation(
                    frelu_sb[:, ht, :], ft[:], Relu, bias=bias_sb[:, ht : ht + 1]
                )
                for jt in range(JT):
                    if ht == 0:
                        psos.append(
                            psump.tile([128, NB], f32, tag="pb", name=f"pso{jt}")
                        )
                    nc.tensor.matmul(
                        psos[jt][:],
                        w1_sb[:, ht, jt * 128 : (jt + 1) * 128],
                        frelu_sb[:, ht, :],
                        start=(ht == 0),
                        stop=(ht == HT - 1),
                    )
            for jt in range(JT):
                ot = evacp.tile([128, NB], f32, tag="ev")
                nc.scalar.activation(
                    ot[:], psos[jt][:], Identity, bias=b1_sb[:, jt : jt + 1]
                )
                nc.vector.tensor_add(ot[:], ot[:], inpsR_sb[:, jt, :])
                nc.sync.dma_start(outT[jt * 128 : (jt + 1) * 128, :], ot[:])

    nc.compile()
    nc.finalize()
    _BUILT["nc"] = nc
    return nc


def _round_fp32r(a):
    """Round fp32 to the fp32r (TF32-like, 1s+8e+11m in top 20 bits) format
    with round-to-nearest-even, as the PE's fp32r datapath expects."""
    b = np.ascontiguousarray(a, np.float32).view(np.uint32).astype(np.uint64)
    lsb = (b >> 12) & 1
    r = ((b + 0x7FF + lsb) & 0xFFFFF000).astype(np.uint32)
    return r.view(np.float32)


def _make_in_maps(inps, fw_adjs, bw_adjs, W_fw, b_fw, W_bw, b_bw, W1, b1):
    f = np.float32
    inps = np.asarray(inps, f)
    W1 = _round_fp32r(np.asarray(W1, f))
    wst = np.ascontiguousarray(
        np.concatenate([np.asarray(W_bw, f), np.asarray(W_fw, f)], axis=0),
        np.float16,
    )
    b_cat = np.concatenate([np.asarray(b_bw, f), np.asarray(b_fw, f)], axis=1)  # [R, H]
    bstack = np.ascontiguousarray(b_cat.T.reshape(4, 128, R))
    b1s = np.ascontiguousarray(np.asarray(b1, f).reshape(4, 128, 1))
    fw_adjs = np.asarray(fw_adjs, f)
    bw_adjs = np.asarray(bw_adjs, f)

    in_maps = []
    for c in range(NC):
        sl = slice(c * NB, (c + 1) * NB)
        adjT_c = np.empty((2 * R, NB, N), np.float16)
        for r in range(R):
            adjT_c[r] = bw_adjs[r][:, sl].T
            adjT_c[R + r] = fw_adjs[r][:, sl].T
        in_maps.append(
            {
                "inpsT": np.ascontiguousarray(inps[sl].T, np.float16),
                "inpsR": np.ascontiguousarray(inps[sl].T),
                "adjT": adjT_c,
                "wst": wst,
                "bstack": bstack,
                "w1": W1,
                "b1s": b1s,
            }
        )
    return in_maps


def run(trace=False, **inputs):
    """Run the SPMD kernel; returns (full_output, BassKernelResults)."""
    from concourse.bass_utils import run_bass_kernel_spmd

    nc = _build_nc()
    in_maps = _make_in_maps(**inputs)
    res = run_bass_kernel_spmd(nc, in_maps, core_ids=list(range(NC)), trace=trace)
    out = np.empty((N, H), np.float32)
    for c in range(NC):
        out[c * NB : (c + 1) * NB] = res.results[c]["outT"].T
    return out, res


def kernel(**inputs):
    # Collective-heavy SPMD runs have shown a rare corrupted execution
    # (launch-skew related). Executions are cheap next to compile, so run
    # twice and accept only agreeing results, with a third as tiebreaker.
    out1, _ = run(trace=False, **inputs)
    out2, _ = run(trace=False, **inputs)
    if np.array_equal(out1, out2):
        return out1
    out3, _ = run(trace=False, **inputs)
    return out3 if np.array_equal(out2, out3) else out1


# revision 54
# speedup vs baseline: 1.0326x; 1.0326x over previous
"""BiGCN layer kernel for 8 Trainium2 NeuronCores.

Strategy (1D column-parallel SpMM, fp16 streams, ReduceScatter epilogue):
  - Each core c owns the contraction slice n in [c*512, (c+1)*512) of all six
    adjacency matrices (3 bw + 3 fw), pre-transposed on host to [n_loc, m]
    fp16 so the contraction dim lands on SBUF partitions with no on-chip
    transposes. fp16 halves the dominant HBM traffic; its 11-bit mantissa
    matches the fp32r (TF32-like) precision class for these [0,1) values.
  - sup[r] = inps @ W[r] is computed locally per core for its n-slice only
    (no support gather: the column-parallel form needs just the local slice,
    so the streams start with zero collective dependency and PJRT launch
    skew + the kernel entry barrier hide under productive work).
  - feats^T partials (all m, summed over a direction's 3 relations directly
    in PSUM) stage to DRAM in fp16 and ReduceScatter across the 8 cores;
    RS(bw) overlaps the fw stream. Core c receives its own m-block.
  - bias+relu fuse into one scalar-engine activation (bias is per-partition
    because feats is produced transposed [h, m]); the final linear runs in
    fp32r, split so its bw half overlaps RS(fw); the residual adds an exact
    fp32 copy of inps^T. Host assembles the 8 transposed output blocks.
"""

import numpy as np

N, H, R = 4096, 512, 3
K = H // 2            # 256
NC = 8                # cores
NB = N // NC          # 512 rows (m / n_loc) per core
MC = 1024             # m-chunk width streamed per PSUM accumulation group

_BUILT = {}


def _build_nc():
    """Build (and cache) the Bass program. Identical program on all 8 cores."""
    if "nc" in _BUILT:
        return _BUILT["nc"]

    import concourse.bass as bass
    import concourse.mybir as mybir
    from concourse import bacc, tile

    f32 = mybir.dt.float32
    f32r = mybir.dt.float32r
    f16 = mybir.dt.float16
    nc = bacc.Bacc(None, num_devices=NC)

    inpsT = nc.dram_tensor("inpsT", [H, NB], f16, kind="ExternalInput")
    inpsR = nc.dram_tensor("inpsR", [H, NB], f32, kind="ExternalInput")
    adjT = nc.dram_tensor("adjT", [2 * R, NB, N], f16, kind="ExternalInput")
    wst = nc.dram_tensor("wst", [2 * R, H, K], f16, kind="ExternalInput")
    bstack = nc.dram_tensor("bstack", [4, 128, R], f32, kind="ExternalInput")
    w1 = nc.dram_tensor("w1", [H, H], f32r, kind="ExternalInput")
    b1s = nc.dram_tensor("b1s", [4, 128, 1], f32, kind="ExternalInput")
    outT = nc.dram_tensor("outT", [H, NB], f32, kind="ExternalOutput")

    HT = H // 128     # 4 h-tiles
    NT = NB // 128    # 4 n_loc tiles
    JT = H // 128     # 4 output j tiles
    NMC = N // MC     # 4 m chunks
    Relu = mybir.ActivationFunctionType.Relu
    Identity = mybir.ActivationFunctionType.Identity

    with tile.TileContext(nc) as tc:
        with (
            tc.tile_pool(name="const", bufs=1) as const,
            tc.tile_pool(name="adjp", bufs=6) as adjp,
            tc.tile_pool(name="evacp", bufs=3) as evacp,
            tc.tile_pool(name="psum", bufs=4, space=bass.MemorySpace.PSUM) as psump,
            tc.tile_pool(name="dram", bufs=1, space="DRAM") as dramp,
        ):
            # ---------------- constants into SBUF ----------------
            inpsT_sb = const.tile([128, HT, NB], f16)       # [p_h, ht, n_loc]
            nc.sync.dma_start(inpsT_sb[:], inpsT[:, :].rearrange("(t p) n -> p t n", p=128))
            wst_sb = const.tile([128, 2 * R, HT, K], f16)   # [p_h, r, ht, k]
            nc.sync.dma_start(wst_sb[:], wst[:, :, :].rearrange("r (t p) k -> p r t k", p=128))
            inpsR_sb = const.tile([128, HT, NB], f32)       # exact fp32 for residual
            nc.scalar.dma_start(inpsR_sb[:], inpsR[:, :].rearrange("(t p) n -> p t n", p=128))
            w1_sb = const.tile([128, HT, H], f32r)          # [p_h, ht, j]
            nc.scalar.dma_start(w1_sb[:], w1[:, :].rearrange("(t p) j -> p t j", p=128))
            bst_sb = const.tile([128, JT, R], f32)
            nc.scalar.dma_start(bst_sb[:], bstack[:, :, :].rearrange("t p r -> p t r"))
            b1_sb = const.tile([128, JT], f32)
            nc.scalar.dma_start(b1_sb[:], b1s[:, :, :].rearrange("t p o -> p (t o)"))

            # summed (over relations) concat bias, per (p, jt)
            bias_sb = const.tile([128, JT], f32)
            for jt in range(JT):
                nc.vector.tensor_add(
                    bias_sb[:, jt : jt + 1], bst_sb[:, jt, 0:1], bst_sb[:, jt, 1:2]
                )
                nc.vector.tensor_add(
                    bias_sb[:, jt : jt + 1], bias_sb[:, jt : jt + 1], bst_sb[:, jt, 2:3]
                )

            # ---------------- local supports: sup[r][n_loc, k] ----------------
            sup_sb = const.tile([128, 2 * R, NT, K], f16)   # [p_n, r, nt, k]
            for r in range(2 * R):
                for nt in range(NT):
                    ps = psump.tile([128, K], f32, tag="pb")
                    for ht in range(HT):
                        nc.tensor.matmul(
                            ps[:],
                            inpsT_sb[:, ht, nt * 128 : (nt + 1) * 128],
                            wst_sb[:, r, ht, :],
                            start=(ht == 0),
                            stop=(ht == HT - 1),
                        )
                    nc.vector.tensor_copy(sup_sb[:, r, nt, :], ps[:])

            # ---------------- adjacency stream + RS staging ----------------
            # One staging tensor per (direction, k-half): separate tensors keep
            # later streams' writes from serializing behind earlier collectives'
            # reads, and quarter-granular RS lets phase C start on each h-row
            # block as soon as its collective lands instead of waiting for a
            # full direction.
            stags = [
                dramp.tile([NC, K, NB], f16, name=f"stag{q}", tag=f"stag{q}")
                for q in range(2)
            ]
            rs_out = [
                dramp.tile([1, K, NB], f16, name=f"rs_out{q}", tag=f"rs_out{q}")
                for q in range(2)
            ]
            for dirn in range(2):                           # 0 = bw (h 0:256), 1 = fw
                for mc in range(NMC):
                    ps0 = psump.tile([128, MC], f32, tag="pb", name="ps0")  # k 0:128
                    ps1 = psump.tile([128, MC], f32, tag="pb", name="ps1")  # k 128:256
                    for ri in range(R):
                        r = dirn * R + ri
                        at = adjp.tile([128, NT, MC], f16, tag="adj")
                        nc.sync.dma_start(
                            at[:],
                            adjT[r, :, mc * MC : (mc + 1) * MC].rearrange(
                                "(t p) m -> p t m", p=128
                            ),
                        )
                        for nt in range(NT):
                            first = ri == 0 and nt == 0
                            last = ri == R - 1 and nt == NT - 1
                            for kk, ps in ((0, ps0), (1, ps1)):
                                lhsT = sup_sb[:, r, nt, kk * 128 : (kk + 1) * 128]
                                for mh in range(MC // 512):
                                    nc.tensor.matmul(
                                        ps[:, mh * 512 : (mh + 1) * 512],
                                        lhsT,
                                        at[:, nt, mh * 512 : (mh + 1) * 512],
                                        start=first,
                                        stop=last,
                                    )
                    for kk, ps in ((0, ps0), (1, ps1)):
                        ev = evacp.tile([128, MC], f16, tag="ev")
                        nc.vector.tensor_copy(ev[:], ps[:])
                        for d2 in range(MC // NB):
                            dest = (mc * MC) // NB + d2
                            nc.scalar.dma_start(
                                stags[dirn][dest, kk * 128 : (kk + 1) * 128, :],
                                ev[:, d2 * NB : (d2 + 1) * NB],
                            )
                nc.gpsimd.collective_compute(
                    "ReduceScatter",
                    mybir.AluOpType.add,
                    replica_groups=[list(range(NC))],
                    ins=[stags[dirn][:].opt()],
                    outs=[rs_out[dirn][:].opt()],
                )

            # ---------------- bias + relu + final linear + residual ----------------
            # The final matmul accumulates per h-row block so each block's
            # matmuls run as soon as its quarter-RS lands, overlapping the
            # remaining collectives.
            frelu_sb = const.tile([128, HT, NB], f32r)      # [p_h, ht, m_loc]
            psos = []
            for ht in range(HT):                            # ht -> (dir, k-half)
                ft = evacp.tile([128, NB], f16, tag="ftmp")
                nc.scalar.dma_start(
                    ft[:], rs_out[ht // 2][0, (ht % 2) * 128 : (ht % 2 + 1) * 128, :]
                )
                nc.scalar.activation(
                    frelu_sb[:, ht, :], ft[:], Relu, bias=bias_sb[:, ht : ht + 1]
                )
                for jt in range(JT):
                    if ht == 0:
                        psos.append(
                            psump.tile([128, NB], f32, tag="pb", name=f"pso{jt}")
                        )
                    nc.tensor.matmul(
                        psos[jt][:],
                        w1_sb[:, ht, jt * 128 : (jt + 1) * 128],
                        frelu_sb[:, ht, :],
                        start=(ht == 0),
                        stop=(ht == HT - 1),
                    )
            for jt in range(JT):
                ot = evacp.tile([128, NB], f32, tag="ev")
                nc.scalar.activation(
                    ot[:], psos[jt][:], Identity, bias=b1_sb[:, jt : jt + 1]
                )
                nc.vector.tensor_add(ot[:], ot[:], inpsR_sb[:, jt, :])
                nc.sync.dma_start(outT[jt * 128 : (jt + 1) * 128, :], ot[:])

    nc.compile()
    nc.finalize()
    _BUILT["nc"] = nc
    return nc


def _round_fp32r(a):
    """Round fp32 to the fp32r (TF32-like, 1s+8e+11m in top 20 bits) format
    with round-to-nearest-even, as the PE's fp32r datapath expects."""
    b = np.ascontiguousarray(a, np.float32).view(np.uint32).astype(np.uint64)
    lsb = (b >> 12) & 1
    r = ((b + 0x7FF + lsb) & 0xFFFFF000).astype(np.uint32)
    return r.view(np.float32)


def _make_in_maps(inps, fw_adjs, bw_adjs, W_fw, b_fw, W_bw, b_bw, W1, b1):
    f = np.float32
    inps = np.asarray(inps, f)
    W1 = _round_fp32r(np.asarray(W1, f))
    wst = np.ascontiguousarray(
        np.concatenate([np.asarray(W_bw, f), np.asarray(W_fw, f)], axis=0),
        np.float16,
    )
    b_cat = np.concatenate([np.asarray(b_bw, f), np.asarray(b_fw, f)], axis=1)  # [R, H]
    bstack = np.ascontiguousarray(b_cat.T.reshape(4, 128, R))
    b1s = np.ascontiguousarray(np.asarray(b1, f).reshape(4, 128, 1))
    fw_adjs = np.asarray(fw_adjs, f)
    bw_adjs = np.asarray(bw_adjs, f)

    in_maps = []
    for c in range(NC):
        sl = slice(c * NB, (c + 1) * NB)
        adjT_c = np.empty((2 * R, NB, N), np.float16)
        for r in range(R):
            adjT_c[r] = bw_adjs[r][:, sl].T
            adjT_c[R + r] = fw_adjs[r][:, sl].T
        in_maps.append(
            {
                "inpsT": np.ascontiguousarray(inps[sl].T, np.float16),
                "inpsR": np.ascontiguousarray(inps[sl].T),
                "adjT": adjT_c,
                "wst": wst,
                "bstack": bstack,
                "w1": W1,
                "b1s": b1s,
            }
        )
    return in_maps


def run(trace=False, **inputs):
    """Run the SPMD kernel; returns (full_output, BassKernelResults)."""
    from concourse.bass_utils import run_bass_kernel_spmd

    nc = _build_nc()
    in_maps = _make_in_maps(**inputs)
    res = run_bass_kernel_spmd(nc, in_maps, core_ids=list(range(NC)), trace=trace)
    out = np.empty((N, H), np.float32)
    for c in range(NC):
        out[c * NB : (c + 1) * NB] = res.results[c]["outT"].T
    return out, res


def kernel(**inputs):
    # Collective-heavy SPMD runs have shown a rare corrupted execution
    # (launch-skew related). Executions are cheap next to compile, so run
    # twice and accept only agreeing results, with a third as tiebreaker.
    out1, _ = run(trace=False, **inputs)
    out2, _ = run(trace=False, **inputs)
    if np.array_equal(out1, out2):
        return out1
    out3, _ = run(trace=False, **inputs)
    return out3 if np.array_equal(out2, out3) else out1
